# revision 9
# baseline (speedup 1.0000x reference)
"""Multi-head attention Trainium2 kernel (B=2, S=4096, D=512, H=8).

Sharding: 8 cores = (batch b = c//4) x (query chunk qs = c%4 of 1024 rows).
Each core computes, for its 1024 query rows: full K/V projections for its
batch, scores/softmax/PV for all 8 heads, and the output projection.
No collectives needed.

v4 strategy (per core):
  - fp16 matmuls on the scores path (q/k), bf16 on v/probs/output path.
    Host-measured max rel err ~4e-3 (gate is 2e-2).
  - The wall is ScalarE: 256 exp ACTIVATEs over [128,1024] f32 PSUM tiles
    at ~1.1us each (PSUM read is 1 elem/cyc/partition) = ~285us. Phase B
    is paced by it; everything else hides underneath:
      * K^T projection for d-blocks 1-3 is INJECTED into phase B's PE
        stream (the PE has ~15% slack under the ACT pace).
      * softmax normalization is split into 3 deferred parts so the
        serial DVE reciprocal never blocks the in-order PE queue:
        part1 (+2 steps): copy pv rows out (releases the PSUM tile),
        part2 (+8): DVE reciprocal, part3 (+14): GpSimd partition
        broadcast + multiply (idle engine, off the DVE).
      * PV psum tile is single-buffered; released at part1 -> no
        head-boundary stall, PE stays HAM-warm (cold PE also slows
        ACT's PSUM reads ~2x).
  - ~10us of dummy warm-up matmuls run under the first DMAs so the HAM
    clock gate opens before real work.
  - PSUM banks: scores ring 2x[128,1024] (4) + pv [65,1024] (2) +
    injected-K-proj ring 2x[128,512] (2) = 8 exactly.
"""

import numpy as np
import ml_dtypes

import bass_rust
import concourse.bass as bass
import concourse.mybir as mybir
from concourse.bass_utils import run_bass_kernel_spmd
from concourse.tile import TileContext

f32 = mybir.dt.float32
f16 = mybir.dt.float16
bf16 = mybir.dt.bfloat16
AF = mybir.ActivationFunctionType
MULT = mybir.AluOpType.mult
ADD = mybir.AluOpType.add

B, S, D, H, HD = 2, 4096, 512, 8, 64
QC = 1024          # query rows per core
NC = 8             # cores
NKB = S // 128     # 32 k-blocks
NDB = D // 128     # 4 d-blocks

_waitfix = [0]


def _legalize_waits(nc):
    """This walrus build accepts at most one sem-wait per instruction.
    Hoist extra waits onto same-engine NOPs inserted just before."""
    for fn in nc.m.functions:
        for bb in fn.blocks:
            out, changed = [], False
            for inst in bb.instructions:
                si = inst.sync_info
                if si is not None and len(si.on_wait) > 1:
                    waits = list(si.on_wait)
                    for w in waits[:-1]:
                        _waitfix[0] += 1
                        nop = mybir.InstNoOp(
                            name=f"I-waitfix-{_waitfix[0]}", ins=[], outs=[])
                        nop.engine = inst.engine
                        nop.sync_info = bass_rust.SyncInfo(on_wait=[w], on_update=[])
                        out.append(nop)
                    inst.sync_info = bass_rust.SyncInfo(
                        on_wait=[waits[-1]], on_update=list(si.on_update))
                    changed = True
                out.append(inst)
            if changed:
                bb.instructions = out


def _build_program(debug_taps=False):
    nc = bass.Bass(target_bir_lowering=False, debug=False)

    xqT = nc.dram_tensor("xqT", [D, QC], f16, kind="ExternalInput")
    xkT = nc.dram_tensor("xkT", [D, S], f16, kind="ExternalInput")
    xvT = nc.dram_tensor("xvT", [D, S], bf16, kind="ExternalInput")
    masktd = nc.dram_tensor("masktd", [S, QC], bf16, kind="ExternalInput")
    wqT = nc.dram_tensor("wqT", [D, D], f16, kind="ExternalInput")
    wkT = nc.dram_tensor("wkT", [D, D], f16, kind="ExternalInput")
    wvT = nc.dram_tensor("wvT", [D, D], bf16, kind="ExternalInput")
    woT = nc.dram_tensor("woT", [D, D], bf16, kind="ExternalInput")
    bq_d = nc.dram_tensor("bq_d", [128, NDB], f32, kind="ExternalInput")
    bk_d = nc.dram_tensor("bk_d", [128, NDB], f32, kind="ExternalInput")
    bvb_d = nc.dram_tensor("bvb_d", [128, D], f32, kind="ExternalInput")
    bob_d = nc.dram_tensor("bob_d", [128, D], f32, kind="ExternalInput")
    outp = nc.dram_tensor("out", [QC, D], f32, kind="ExternalOutput")
    if debug_taps:
        dbg_qt = nc.dram_tensor("dbg_qt", [NDB, 128, QC], f32, kind="ExternalOutput")
        dbg_kt = nc.dram_tensor("dbg_kt", [NDB, 128, S], f32, kind="ExternalOutput")
        dbg_va = nc.dram_tensor("dbg_va", [NKB, 128, 520], f32, kind="ExternalOutput")
        dbg_at = nc.dram_tensor("dbg_at", [NDB, 128, QC], f32, kind="ExternalOutput")

    with TileContext(nc) as tc:
        with tc.tile_pool(name="cpool", bufs=1) as cpool, \
             tc.tile_pool(name="psAB", bufs=1, space="PSUM") as psB:
            # ---- constants ----
            ones_f = cpool.tile([1, 128], f32, tag="ones_f")
            nc.vector.memset(ones_f[:, :], 1.0)
            ones_b = cpool.tile([1, 128], bf16, tag="ones_b")
            nc.vector.tensor_copy(ones_b[:, :], ones_f[:, :])
            bq_t = cpool.tile([128, NDB], f32, tag="bq")
            bk_t = cpool.tile([128, NDB], f32, tag="bk")
            bvb = cpool.tile([128, D], f32, tag="bvb")
            bob = cpool.tile([128, D], f32, tag="bob")
            nc.scalar.dma_start(out=bq_t[:, :], in_=bq_d[:, :])
            nc.scalar.dma_start(out=bk_t[:, :], in_=bk_d[:, :])
            nc.scalar.dma_start(out=bvb[:, :], in_=bvb_d[:, :])
            nc.scalar.dma_start(out=bob[:, :], in_=bob_d[:, :])
            wo_p = []
            for hp in range(NDB):
                t = cpool.tile([128, D], bf16, tag=f"wo{hp}", name=f"wo{hp}")
                nc.scalar.dma_start(out=t[:, :], in_=woT[hp * 128:(hp + 1) * 128, :])
                wo_p.append(t)
            # K weights persist into phase B (injected K projection)
            wink = [cpool.tile([128, D], f16, tag=f"wink{kc}", name=f"wink{kc}")
                    for kc in range(NDB)]
            for kc in range(NDB):
                nc.scalar.dma_start(out=wink[kc][:, :],
                                    in_=wkT[kc * 128:(kc + 1) * 128, :])
            # persistent per-core state
            qt = [cpool.tile([128, QC], f16, tag=f"qt{db}", name=f"qt{db}")
                  for db in range(NDB)]
            kt = [cpool.tile([128, S], f16, tag=f"kt{db}", name=f"kt{db}")
                  for db in range(NDB)]
            va = [cpool.tile([128, 8 * 65], bf16, tag=f"va{sb}", name=f"va{sb}")
                  for sb in range(NKB)]
            mk = [cpool.tile([128, QC], bf16, tag=f"mk{kb}", name=f"mk{kb}")
                  for kb in range(NKB)]
            at4 = [cpool.tile([128, QC], bf16, tag=f"at{hp}", name=f"at{hp}")
                   for hp in range(NDB)]

            # PE warm-up: dummy matmuls while the first DMAs land, so the
            # HAM clock-gate opens before real work starts.
            wsrc = cpool.tile([128, 512], bf16, tag="wsrc")
            nc.vector.memset(wsrc[:, :], 0.0)
            wps = psB.tile([128, QC], f32, tag="sc", bufs=2, name="warm")
            for i in range(48):
                nc.tensor.matmul(wps[:, 0:512], wsrc[:, 0:128], wsrc[:, :],
                                 start=True, stop=True)

            # ================= PHASE A: Q proj, V proj, K proj d-block 0 ====
            with tc.tile_pool(name="apool", bufs=1) as apool:
                xq = [apool.tile([128, QC], f16, tag=f"xq{kc}", name=f"xq{kc}")
                      for kc in range(NDB)]
                for kc in range(NDB):
                    nc.sync.dma_start(out=xq[kc][:, :],
                                      in_=xqT[kc * 128:(kc + 1) * 128, :])
                winq = [apool.tile([128, D], f16, tag=f"winq{kc}", name=f"winq{kc}")
                        for kc in range(NDB)]
                winv = [apool.tile([128, D], bf16, tag=f"winv{kc}", name=f"winv{kc}")
                        for kc in range(NDB)]
                for kc in range(NDB):
                    nc.sync.dma_start(out=winq[kc][:, :],
                                      in_=wqT[kc * 128:(kc + 1) * 128, :])
                    nc.scalar.dma_start(out=winv[kc][:, :],
                                        in_=wvT[kc * 128:(kc + 1) * 128, :])
                for db in range(NDB):
                    ps = psB.tile([128, QC], f32, tag="sc", bufs=2)
                    for ch in range(2):
                        for kc in range(NDB):
                            nc.tensor.matmul(
                                ps[:, ch * 512:(ch + 1) * 512],
                                winq[kc][:, db * 128:(db + 1) * 128],
                                xq[kc][:, ch * 512:(ch + 1) * 512],
                                start=(kc == 0), stop=(kc == NDB - 1))
                    nc.vector.tensor_scalar_add(qt[db][:, :], ps[:, :],
                                                bq_t[:, db:db + 1])

                for q4 in range(4):
                    xin_k = [apool.tile([128, QC], f16, tag=f"xink{kc}", bufs=2,
                                        name=f"xink{q4}{kc}") for kc in range(NDB)]
                    xin_v = [apool.tile([128, QC], bf16, tag=f"xinv{kc}", bufs=2,
                                        name=f"xinv{q4}{kc}") for kc in range(NDB)]
                    for kc in range(NDB):
                        nc.sync.dma_start(
                            out=xin_k[kc][:, :],
                            in_=xkT[kc * 128:(kc + 1) * 128,
                                    q4 * QC:(q4 + 1) * QC])
                        nc.scalar.dma_start(
                            out=xin_v[kc][:, :],
                            in_=xvT[kc * 128:(kc + 1) * 128,
                                    q4 * QC:(q4 + 1) * QC])
                    for kb in range(q4 * 8, q4 * 8 + 8):
                        nc.scalar.dma_start(out=mk[kb][:, :],
                                            in_=masktd[kb * 128:(kb + 1) * 128, :])
                    # K^T d-block 0 for this q4
                    ps = psB.tile([128, QC], f32, tag="sc", bufs=2)
                    for ch in range(2):
                        for kc in range(NDB):
                            nc.tensor.matmul(
                                ps[:, ch * 512:(ch + 1) * 512],
                                wink[kc][:, 0:128],
                                xin_k[kc][:, ch * 512:(ch + 1) * 512],
                                start=(kc == 0), stop=(kc == NDB - 1))
                    nc.vector.tensor_scalar_add(
                        kt[0][:, q4 * QC:(q4 + 1) * QC], ps[:, :],
                        bk_t[:, 0:1])
                    # V natural [s, d] -> va bf16 + ones col
                    for sbl in range(8):
                        sb = q4 * 8 + sbl
                        psv = psB.tile([128, QC], f32, tag="sc", bufs=2,
                                       name="vps")[:, 0:512]
                        for kc in range(NDB):
                            nc.tensor.matmul(
                                psv[:, :],
                                xin_v[kc][:, sbl * 128:(sbl + 1) * 128],
                                winv[kc][:, :],
                                start=(kc == 0), stop=(kc == NDB - 1))
                        dst = va[sb][:, :].rearrange("p (h c) -> p h c", c=65)
                        src2 = psv[:, :].rearrange("p (h c) -> p h c", c=64)
                        bsr = bvb[:, :].rearrange("p (h c) -> p h c", c=64)
                        nc.vector.tensor_tensor(dst[:, :, 0:64], src2[:, :, :],
                                                bsr[:, :, :], op=ADD)
                        nc.vector.memset(dst[:, :, 64:65], 1.0)

            if debug_taps:
                with tc.tile_pool(name="dbgp", bufs=1) as dbgp:
                    for db in range(NDB):
                        t = dbgp.tile([128, QC], f32, tag="dq", bufs=2)
                        nc.vector.tensor_copy(t[:, :], qt[db][:, :])
                        nc.sync.dma_start(out=dbg_qt[db, :, :], in_=t[:, :])
                    for sb in range(NKB):
                        t = dbgp.tile([128, 520], f32, tag="dv", bufs=2)
                        nc.vector.tensor_copy(t[:, :], va[sb][:, :])
                        nc.sync.dma_start(out=dbg_va[sb, :, :], in_=t[:, :])

            # ================= PHASE B: attention + injected K proj =========
            with tc.tile_pool(name="bpool", bufs=1) as bpool:
              LAG = 4        # PV matmul trails scores by LAG steps
              P1, P2, P3 = 2, 8, 14   # norm pipeline delays after head end
              steps = [(hp, hh, kb) for hp in range(4) for hh in range(2)
                       for kb in range(NKB)]
              pvt = {}
              pend_pv = []       # (h, kb, probs_tile)
              norm_q = []        # [due_step, stage, h, state]

              # ---- injected K projection schedule (db = hp+1 during hp) ----
              # per (db, q4): 4 xkb DMAs, then 2 half-blocks of 4 MMs + bias
              inj_dma = {}   # step -> list of (db, q4)
              inj_mm = {}    # step -> list of (db, q4, half)
              for db in (1, 2, 3):
                  base = (db - 1) * 64
                  for q4 in range(4):
                      inj_dma.setdefault(base + max(0, 16 * q4 - 10), []).append(
                          (db, q4))
                      for half in range(2):
                          inj_mm.setdefault(base + 16 * q4 + 6 + 6 * half,
                                            []).append((db, q4, half))
              xkb = {}

              def emit_pv(h, kb, probs):
                  for ch in range(2):
                      nc.tensor.matmul(
                          pvt[h][:, ch * 512:(ch + 1) * 512],
                          va[kb][:, h * 65:(h + 1) * 65],
                          probs[:, ch * 512:(ch + 1) * 512],
                          start=(kb == 0), stop=(kb == NKB - 1))

              def norm_part1(h):
                  pv = pvt[h]
                  stg = bpool.tile([64, QC], f32, tag="stg", bufs=2,
                                   name=f"stg{h}")
                  nc.vector.tensor_copy(stg[:, :], pv[0:64, :])
                  denc = bpool.tile([1, QC], bf16, tag="denc", bufs=2,
                                    name=f"denc{h}")
                  nc.vector.tensor_copy(denc[:, :], pv[64:65, :])
                  return (stg, denc)

              def norm_part2(h, st):
                  stg, denc = st
                  rden = bpool.tile([1, QC], bf16, tag="rden", bufs=2,
                                    name=f"rden{h}")
                  with nc.allow_low_precision(reason="softmax denom recip"):
                      nc.vector.reciprocal(rden[:, :], denc[:, :])
                  return (stg, rden)

              def norm_part3(h, st):
                  stg, rden = st
                  ps_n = psB.tile([128, QC], f32, tag="sc", bufs=2,
                                  name=f"nrm{h}")
                  for ch in range(2):
                      nc.tensor.matmul(ps_n[0:64, ch * 512:(ch + 1) * 512],
                                       ones_b[0:1, 0:64],
                                       rden[0:1, ch * 512:(ch + 1) * 512],
                                       start=True, stop=True)
                  hp, hh = divmod(h, 2)
                  nc.vector.tensor_tensor(at4[hp][hh * 64:(hh + 1) * 64, :],
                                          stg[:, :], ps_n[0:64, :], op=MULT)

              def norm_advance(t, nq):
                  while nq and nq[0][0] <= t:
                      due, stage, nh, st = nq.pop(0)
                      if stage == 1:
                          nq.append([due + (P2 - P1), 2, nh, norm_part1(nh)])
                      elif stage == 2:
                          nq.append([due + (P3 - P2), 3, nh, norm_part2(nh, st)])
                      else:
                          norm_part3(nh, st)
                      nq.sort(key=lambda e: e[0])

              for t, (hp, hh, kb) in enumerate(steps):
                  h = hp * 2 + hh
                  if kb == 0:
                      pvt[h] = psB.tile([65, QC], f32, tag="pv", bufs=1,
                                        name=f"pv{h}")
                  # injected K-proj DMAs for upcoming blocks
                  for (db, q4) in inj_dma.get(t, ()):
                      tiles = [bpool.tile([128, QC], f16, tag=f"xkb{kc}",
                                          bufs=2, name=f"xkb{db}{q4}{kc}")
                               for kc in range(NDB)]
                      for kc in range(NDB):
                          nc.sync.dma_start(
                              out=tiles[kc][:, :],
                              in_=xkT[kc * 128:(kc + 1) * 128,
                                      q4 * QC:(q4 + 1) * QC])
                      xkb[(db, q4)] = tiles
                  # injected K-proj matmuls
                  for (db, q4, half) in inj_mm.get(t, ()):
                      tiles = xkb[(db, q4)]
                      pj = psB.tile([128, 512], f32, tag="pj", bufs=2)
                      for kc in range(NDB):
                          nc.tensor.matmul(
                              pj[:, :],
                              wink[kc][:, db * 128:(db + 1) * 128],
                              tiles[kc][:, half * 512:(half + 1) * 512],
                              start=(kc == 0), stop=(kc == NDB - 1))
                      nc.vector.tensor_scalar_add(
                          kt[db][:, q4 * QC + half * 512:
                                 q4 * QC + (half + 1) * 512],
                          pj[:, :], bk_t[:, db:db + 1])
                  # scores
                  ps = psB.tile([128, QC], f32, tag="sc", bufs=2)
                  for ch in range(2):
                      nc.tensor.matmul(
                          ps[:, ch * 512:(ch + 1) * 512],
                          kt[hp][hh * 64:(hh + 1) * 64,
                                 kb * 128:(kb + 1) * 128],
                          qt[hp][hh * 64:(hh + 1) * 64,
                                 ch * 512:(ch + 1) * 512],
                          start=True, stop=True)
                  probs = bpool.tile([128, QC], bf16, tag="probs", bufs=6)
                  nc.scalar.activation(probs[:, :], ps[:, :], AF.Exp)
                  nc.vector.tensor_tensor(probs[:, :], probs[:, :],
                                          mk[kb][:, :], op=MULT)
                  norm_advance(t, norm_q)
                  # PV trailing
                  pend_pv.append((h, kb, probs))
                  if len(pend_pv) > LAG:
                      ph, pkb, pprobs = pend_pv.pop(0)
                      emit_pv(ph, pkb, pprobs)
                      if pkb == NKB - 1:
                          norm_q.append([t + P1, 1, ph, None])
                          norm_q.sort(key=lambda e: e[0])
              # drain
              t = len(steps)
              while pend_pv:
                  ph, pkb, pprobs = pend_pv.pop(0)
                  emit_pv(ph, pkb, pprobs)
                  if pkb == NKB - 1:
                      norm_q.append([t + P1, 1, ph, None])
                      norm_q.sort(key=lambda e: e[0])
              norm_advance(10 ** 9, norm_q)
              if debug_taps:
                  for hp in range(NDB):
                      tdb2 = bpool.tile([128, QC], f32, tag="dbg", bufs=2)
                      nc.vector.tensor_copy(tdb2[:, :], at4[hp][:, :])
                      nc.sync.dma_start(out=dbg_at[hp, :, :], in_=tdb2[:, :])

              # ================= PHASE C: output projection ==================
              for sb in range(8):
                  po = psB.tile([128, QC], f32, tag="sc", bufs=2,
                                name="po")[:, 0:512]
                  for hp in range(NDB):
                      nc.tensor.matmul(
                          po[:, :],
                          at4[hp][:, sb * 128:(sb + 1) * 128],
                          wo_p[hp][:, :],
                          start=(hp == 0), stop=(hp == NDB - 1))
                  osb = bpool.tile([128, D], f32, tag="osb", bufs=2)
                  nc.vector.tensor_tensor(osb[:, :], po[:, :], bob[:, :], op=ADD)
                  nc.sync.dma_start(out=outp[sb * 128:(sb + 1) * 128, :],
                                    in_=osb[:, :])

    _legalize_waits(nc)
    return nc


_program_cache = {}
_last_in_maps = None


def _get_program():
    if "nc" not in _program_cache:
        _program_cache["nc"] = _build_program()
    return _program_cache["nc"]


def kernel(query, key, value, mask, Wq, bq, Wk, bk, Wv, bv, Wo, bo, **_unused):
    query = np.asarray(query, dtype=np.float32)
    key = np.asarray(key, dtype=np.float32)
    value = np.asarray(value, dtype=np.float32)
    mask = np.asarray(mask)

    wqT = np.ascontiguousarray(np.asarray(Wq, np.float32).T).astype(np.float16)
    wkT = np.ascontiguousarray(np.asarray(Wk, np.float32).T).astype(np.float16)
    wvT = np.ascontiguousarray(np.asarray(Wv, np.float32).T).astype(ml_dtypes.bfloat16)
    woT = np.ascontiguousarray(np.asarray(Wo, np.float32).T).astype(ml_dtypes.bfloat16)
    bq_h = np.ascontiguousarray(np.asarray(bq, np.float32).reshape(NDB, 128).T)
    bk_h = np.ascontiguousarray(np.asarray(bk, np.float32).reshape(NDB, 128).T)
    # broadcast tiles: bv/bo replicated across all 128 partitions
    bvb_h = np.ascontiguousarray(
        np.broadcast_to(np.asarray(bv, np.float32).reshape(1, D), (128, D)))
    bob_h = np.ascontiguousarray(
        np.broadcast_to(np.asarray(bo, np.float32).reshape(1, D), (128, D)))

    # bf16 bits for the (0/1) mask: exact; pre-transposed per batch
    mbits = (mask != 0).astype(np.uint16) * np.uint16(0x3F80)
    mbitsT = [np.ascontiguousarray(mbits[b].T) for b in range(B)]

    xT = {}
    for b in range(B):
        xT[("q", b)] = np.ascontiguousarray(query[b].T).astype(np.float16)
        xT[("k", b)] = np.ascontiguousarray(key[b].T).astype(np.float16)
        xT[("v", b)] = np.ascontiguousarray(value[b].T).astype(ml_dtypes.bfloat16)

    in_maps = []
    for c in range(NC):
        b, qs = divmod(c, 4)
        in_maps.append({
            "xqT": np.ascontiguousarray(xT[("q", b)][:, qs * QC:(qs + 1) * QC]),
            "xkT": xT[("k", b)],
            "xvT": xT[("v", b)],
            "masktd": np.ascontiguousarray(
                mbitsT[b][:, qs * QC:(qs + 1) * QC]).view(ml_dtypes.bfloat16),
            "wqT": wqT, "wkT": wkT, "wvT": wvT, "woT": woT,
            "bq_d": bq_h, "bk_d": bk_h, "bvb_d": bvb_h, "bob_d": bob_h,
        })

    global _last_in_maps
    _last_in_maps = in_maps
    nc = _get_program()
    res = run_bass_kernel_spmd(nc, in_maps, list(range(NC)))

    out = np.empty((B, S, D), np.float32)
    for c in range(NC):
        b, qs = divmod(c, 4)
        out[b, qs * QC:(qs + 1) * QC, :] = res.results[c]["out"]
    return out


# revision 11
# speedup vs baseline: 1.1027x; 1.1027x over previous
"""Multi-head attention Trainium2 kernel (B=2, S=4096, D=512, H=8).

Sharding: 8 cores = (batch b = c//4) x (query chunk qs = c%4 of 1024 rows).
Each core computes, for its 1024 query rows: full K/V projections for its
batch, scores/softmax/PV for all 8 heads, and the output projection.
No collectives needed.

v4 strategy (per core):
  - fp16 matmuls on the scores path (q/k), bf16 on v/probs/output path.
    Host-measured max rel err ~4e-3 (gate is 2e-2).
  - The wall is ScalarE: 256 exp ACTIVATEs over [128,1024] f32 PSUM tiles
    at ~1.1us each (PSUM read is 1 elem/cyc/partition) = ~285us. Phase B
    is paced by it; everything else hides underneath:
      * K^T projection for d-blocks 1-3 is INJECTED into phase B's PE
        stream (the PE has ~15% slack under the ACT pace).
      * softmax normalization is split into 3 deferred parts so the
        serial DVE reciprocal never blocks the in-order PE queue:
        part1 (+2 steps): copy pv rows out (releases the PSUM tile),
        part2 (+8): DVE reciprocal, part3 (+14): GpSimd partition
        broadcast + multiply (idle engine, off the DVE).
      * PV psum tile is single-buffered; released at part1 -> no
        head-boundary stall, PE stays HAM-warm (cold PE also slows
        ACT's PSUM reads ~2x).
  - ~10us of dummy warm-up matmuls run under the first DMAs so the HAM
    clock gate opens before real work.
  - PSUM banks: scores ring 2x[128,1024] (4) + pv [65,1024] (2) +
    injected-K-proj ring 2x[128,512] (2) = 8 exactly.
"""

import numpy as np
import ml_dtypes

import bass_rust
import concourse.bass as bass
import concourse.mybir as mybir
from concourse.bass_utils import run_bass_kernel_spmd
from concourse.tile import TileContext

f32 = mybir.dt.float32
f16 = mybir.dt.float16
bf16 = mybir.dt.bfloat16
AF = mybir.ActivationFunctionType
MULT = mybir.AluOpType.mult
ADD = mybir.AluOpType.add

B, S, D, H, HD = 2, 4096, 512, 8, 64
QC = 1024          # query rows per core
NC = 8             # cores
NKB = S // 128     # 32 k-blocks
NDB = D // 128     # 4 d-blocks

_waitfix = [0]


def _legalize_waits(nc):
    """This walrus build accepts at most one sem-wait per instruction.
    Hoist extra waits onto same-engine NOPs inserted just before."""
    for fn in nc.m.functions:
        for bb in fn.blocks:
            out, changed = [], False
            for inst in bb.instructions:
                si = inst.sync_info
                if si is not None and len(si.on_wait) > 1:
                    waits = list(si.on_wait)
                    for w in waits[:-1]:
                        _waitfix[0] += 1
                        nop = mybir.InstNoOp(
                            name=f"I-waitfix-{_waitfix[0]}", ins=[], outs=[])
                        nop.engine = inst.engine
                        nop.sync_info = bass_rust.SyncInfo(on_wait=[w], on_update=[])
                        out.append(nop)
                    inst.sync_info = bass_rust.SyncInfo(
                        on_wait=[waits[-1]], on_update=list(si.on_update))
                    changed = True
                out.append(inst)
            if changed:
                bb.instructions = out


def _build_program(debug_taps=False):
    nc = bass.Bass(target_bir_lowering=False, debug=False)

    xqT = nc.dram_tensor("xqT", [D, QC], f16, kind="ExternalInput")
    xkT = nc.dram_tensor("xkT", [D, S], f16, kind="ExternalInput")
    xvT = nc.dram_tensor("xvT", [D, S], bf16, kind="ExternalInput")
    masktd = nc.dram_tensor("masktd", [S, QC], bf16, kind="ExternalInput")
    wqT = nc.dram_tensor("wqT", [D, D], f16, kind="ExternalInput")
    wkT = nc.dram_tensor("wkT", [D, D], f16, kind="ExternalInput")
    wvT = nc.dram_tensor("wvT", [D, D], bf16, kind="ExternalInput")
    woT = nc.dram_tensor("woT", [D, D], bf16, kind="ExternalInput")
    bq_d = nc.dram_tensor("bq_d", [128, NDB], f32, kind="ExternalInput")
    bk_d = nc.dram_tensor("bk_d", [128, NDB], f32, kind="ExternalInput")
    bvb_d = nc.dram_tensor("bvb_d", [128, D], f32, kind="ExternalInput")
    bob_d = nc.dram_tensor("bob_d", [128, D], f32, kind="ExternalInput")
    outp = nc.dram_tensor("out", [QC, D], f32, kind="ExternalOutput")
    if debug_taps:
        dbg_qt = nc.dram_tensor("dbg_qt", [NDB, 128, QC], f32, kind="ExternalOutput")
        dbg_kt = nc.dram_tensor("dbg_kt", [NDB, 128, S], f32, kind="ExternalOutput")
        dbg_va = nc.dram_tensor("dbg_va", [NKB, 128, 520], f32, kind="ExternalOutput")
        dbg_at = nc.dram_tensor("dbg_at", [NDB, 128, QC], f32, kind="ExternalOutput")

    with TileContext(nc) as tc:
        with tc.tile_pool(name="cpool", bufs=1) as cpool, \
             tc.tile_pool(name="psAB", bufs=1, space="PSUM") as psB:
            # ---- constants ----
            ones_f = cpool.tile([1, 128], f32, tag="ones_f")
            nc.vector.memset(ones_f[:, :], 1.0)
            ones_b = cpool.tile([1, 128], bf16, tag="ones_b")
            nc.vector.tensor_copy(ones_b[:, :], ones_f[:, :])
            bq_t = cpool.tile([128, NDB], f32, tag="bq")
            bk_t = cpool.tile([128, NDB], f32, tag="bk")
            bvb = cpool.tile([128, D], f32, tag="bvb")
            bob = cpool.tile([128, D], f32, tag="bob")
            nc.scalar.dma_start(out=bq_t[:, :], in_=bq_d[:, :])
            nc.scalar.dma_start(out=bk_t[:, :], in_=bk_d[:, :])
            nc.scalar.dma_start(out=bvb[:, :], in_=bvb_d[:, :])
            nc.scalar.dma_start(out=bob[:, :], in_=bob_d[:, :])
            wo_p = []
            for hp in range(NDB):
                t = cpool.tile([128, D], bf16, tag=f"wo{hp}", name=f"wo{hp}")
                nc.scalar.dma_start(out=t[:, :], in_=woT[hp * 128:(hp + 1) * 128, :])
                wo_p.append(t)
            # K weights persist into phase B (injected K projection)
            wink = [cpool.tile([128, D], f16, tag=f"wink{kc}", name=f"wink{kc}")
                    for kc in range(NDB)]
            for kc in range(NDB):
                nc.scalar.dma_start(out=wink[kc][:, :],
                                    in_=wkT[kc * 128:(kc + 1) * 128, :])
            # persistent per-core state
            qt = [cpool.tile([128, QC], f16, tag=f"qt{db}", name=f"qt{db}")
                  for db in range(NDB)]
            kt = [cpool.tile([128, S], f16, tag=f"kt{db}", name=f"kt{db}")
                  for db in range(NDB)]
            va = [cpool.tile([128, 8 * 65], bf16, tag=f"va{sb}", name=f"va{sb}")
                  for sb in range(NKB)]
            mk = [cpool.tile([128, QC], bf16, tag=f"mk{kb}", name=f"mk{kb}")
                  for kb in range(NKB)]
            at4 = [cpool.tile([128, QC], bf16, tag=f"at{hp}", name=f"at{hp}")
                   for hp in range(NDB)]

            # PE warm-up: dummy matmuls while the first DMAs land, so the
            # HAM clock-gate opens before real work starts.
            wsrc = cpool.tile([128, 512], bf16, tag="wsrc")
            nc.vector.memset(wsrc[:, :], 0.0)
            wps = psB.tile([128, QC], f32, tag="sc", bufs=2, name="warm")
            for i in range(48):
                nc.tensor.matmul(wps[:, 0:512], wsrc[:, 0:128], wsrc[:, :],
                                 start=True, stop=True)

            # ================= PHASE A: Q proj, V proj, K proj d-block 0 ====
            with tc.tile_pool(name="apool", bufs=1) as apool:
                xq = [apool.tile([128, QC], f16, tag=f"xq{kc}", name=f"xq{kc}")
                      for kc in range(NDB)]
                for kc in range(NDB):
                    nc.sync.dma_start(out=xq[kc][:, :],
                                      in_=xqT[kc * 128:(kc + 1) * 128, :])
                winq = [apool.tile([128, D], f16, tag=f"winq{kc}", name=f"winq{kc}")
                        for kc in range(NDB)]
                winv = [apool.tile([128, D], bf16, tag=f"winv{kc}", name=f"winv{kc}")
                        for kc in range(NDB)]
                for kc in range(NDB):
                    nc.sync.dma_start(out=winq[kc][:, :],
                                      in_=wqT[kc * 128:(kc + 1) * 128, :])
                    nc.scalar.dma_start(out=winv[kc][:, :],
                                        in_=wvT[kc * 128:(kc + 1) * 128, :])
                for db in range(NDB):
                    ps = psB.tile([128, QC], f32, tag="sc", bufs=2)
                    for ch in range(2):
                        for kc in range(NDB):
                            nc.tensor.matmul(
                                ps[:, ch * 512:(ch + 1) * 512],
                                winq[kc][:, db * 128:(db + 1) * 128],
                                xq[kc][:, ch * 512:(ch + 1) * 512],
                                start=(kc == 0), stop=(kc == NDB - 1))
                    nc.vector.tensor_scalar_add(qt[db][:, :], ps[:, :],
                                                bq_t[:, db:db + 1])

                for q4 in range(4):
                    xin_k = [apool.tile([128, QC], f16, tag=f"xink{kc}", bufs=2,
                                        name=f"xink{q4}{kc}") for kc in range(NDB)]
                    xin_v = [apool.tile([128, QC], bf16, tag=f"xinv{kc}", bufs=2,
                                        name=f"xinv{q4}{kc}") for kc in range(NDB)]
                    for kc in range(NDB):
                        nc.sync.dma_start(
                            out=xin_k[kc][:, :],
                            in_=xkT[kc * 128:(kc + 1) * 128,
                                    q4 * QC:(q4 + 1) * QC])
                        nc.scalar.dma_start(
                            out=xin_v[kc][:, :],
                            in_=xvT[kc * 128:(kc + 1) * 128,
                                    q4 * QC:(q4 + 1) * QC])
                    for kb in range(q4 * 8, q4 * 8 + 8):
                        nc.scalar.dma_start(out=mk[kb][:, :],
                                            in_=masktd[kb * 128:(kb + 1) * 128, :])
                    # K^T d-block 0 for this q4
                    ps = psB.tile([128, QC], f32, tag="sc", bufs=2)
                    for ch in range(2):
                        for kc in range(NDB):
                            nc.tensor.matmul(
                                ps[:, ch * 512:(ch + 1) * 512],
                                wink[kc][:, 0:128],
                                xin_k[kc][:, ch * 512:(ch + 1) * 512],
                                start=(kc == 0), stop=(kc == NDB - 1))
                    nc.vector.tensor_scalar_add(
                        kt[0][:, q4 * QC:(q4 + 1) * QC], ps[:, :],
                        bk_t[:, 0:1])
                    # V natural [s, d] -> va bf16 + ones col
                    for sbl in range(8):
                        sb = q4 * 8 + sbl
                        psv = psB.tile([128, QC], f32, tag="sc", bufs=2,
                                       name="vps")[:, 0:512]
                        for kc in range(NDB):
                            nc.tensor.matmul(
                                psv[:, :],
                                xin_v[kc][:, sbl * 128:(sbl + 1) * 128],
                                winv[kc][:, :],
                                start=(kc == 0), stop=(kc == NDB - 1))
                        dst = va[sb][:, :].rearrange("p (h c) -> p h c", c=65)
                        src2 = psv[:, :].rearrange("p (h c) -> p h c", c=64)
                        bsr = bvb[:, :].rearrange("p (h c) -> p h c", c=64)
                        nc.vector.tensor_tensor(dst[:, :, 0:64], src2[:, :, :],
                                                bsr[:, :, :], op=ADD)
                        nc.vector.memset(dst[:, :, 64:65], 1.0)

            if debug_taps:
                with tc.tile_pool(name="dbgp", bufs=1) as dbgp:
                    for db in range(NDB):
                        t = dbgp.tile([128, QC], f32, tag="dq", bufs=2)
                        nc.vector.tensor_copy(t[:, :], qt[db][:, :])
                        nc.sync.dma_start(out=dbg_qt[db, :, :], in_=t[:, :])
                    for sb in range(NKB):
                        t = dbgp.tile([128, 520], f32, tag="dv", bufs=2)
                        nc.vector.tensor_copy(t[:, :], va[sb][:, :])
                        nc.sync.dma_start(out=dbg_va[sb, :, :], in_=t[:, :])

            # ================= PHASE B: attention + injected K proj =========
            with tc.tile_pool(name="bpool", bufs=1) as bpool:
              LAG = 4        # PV matmul trails scores by LAG steps
              P1, P2, P3 = 2, 8, 14   # norm pipeline delays after head end
              steps = [(hp, hh, kb) for hp in range(4) for hh in range(2)
                       for kb in range(NKB)]
              pvt = {}
              pend_pv = []       # (h, kb, probs_tile)
              norm_q = []        # [due_step, stage, h, state]

              # ---- injected K projection schedule (db = hp+1 during hp) ----
              # per (db, q4): 4 xkb DMAs, then 2 half-blocks of 4 MMs + bias
              inj_dma = {}   # step -> list of (db, q4)
              inj_mm = {}    # step -> list of (db, q4, half)
              for db in (1, 2, 3):
                  base = (db - 1) * 64
                  for q4 in range(4):
                      inj_dma.setdefault(base + max(0, 16 * q4 - 10), []).append(
                          (db, q4))
                      for half in range(2):
                          inj_mm.setdefault(base + 16 * q4 + 6 + 6 * half,
                                            []).append((db, q4, half))
              xkb = {}

              def emit_pv(h, kb, probs):
                  for ch in range(2):
                      nc.tensor.matmul(
                          pvt[h][:, ch * 512:(ch + 1) * 512],
                          va[kb][:, h * 65:(h + 1) * 65],
                          probs[:, ch * 512:(ch + 1) * 512],
                          start=(kb == 0), stop=(kb == NKB - 1))

              def norm_part1(h):
                  pv = pvt[h]
                  stg = bpool.tile([65, QC], f32, tag="stg", bufs=2,
                                   name=f"stg{h}")
                  nc.vector.tensor_copy(stg[:, :], pv[0:65, :])
                  return (stg,)

              def norm_part2(h, st):
                  (stg,) = st
                  rden = bpool.tile([1, QC], bf16, tag="rden", bufs=2,
                                    name=f"rden{h}")
                  with nc.allow_low_precision(reason="softmax denom recip"):
                      nc.vector.reciprocal(rden[:, :], stg[64:65, :])
                  return (stg, rden)

              def norm_part3(h, st):
                  stg, rden = st
                  ps_n = psB.tile([128, QC], f32, tag="sc", bufs=2,
                                  name=f"nrm{h}")
                  for ch in range(2):
                      nc.tensor.matmul(ps_n[0:64, ch * 512:(ch + 1) * 512],
                                       ones_b[0:1, 0:64],
                                       rden[0:1, ch * 512:(ch + 1) * 512],
                                       start=True, stop=True)
                  hp, hh = divmod(h, 2)
                  nc.vector.tensor_tensor(at4[hp][hh * 64:(hh + 1) * 64, :],
                                          stg[0:64, :], ps_n[0:64, :], op=MULT)

              def norm_advance(t, nq):
                  while nq and nq[0][0] <= t:
                      due, stage, nh, st = nq.pop(0)
                      if stage == 1:
                          nq.append([due + (P2 - P1), 2, nh, norm_part1(nh)])
                      elif stage == 2:
                          nq.append([due + (P3 - P2), 3, nh, norm_part2(nh, st)])
                      else:
                          norm_part3(nh, st)
                      nq.sort(key=lambda e: e[0])

              for t, (hp, hh, kb) in enumerate(steps):
                  h = hp * 2 + hh
                  if kb == 0:
                      pvt[h] = psB.tile([65, QC], f32, tag="pv", bufs=1,
                                        name=f"pv{h}")
                  # injected K-proj DMAs for upcoming blocks
                  for (db, q4) in inj_dma.get(t, ()):
                      tiles = [bpool.tile([128, QC], f16, tag=f"xkb{kc}",
                                          bufs=2, name=f"xkb{db}{q4}{kc}")
                               for kc in range(NDB)]
                      for kc in range(NDB):
                          nc.sync.dma_start(
                              out=tiles[kc][:, :],
                              in_=xkT[kc * 128:(kc + 1) * 128,
                                      q4 * QC:(q4 + 1) * QC])
                      xkb[(db, q4)] = tiles
                  # injected K-proj matmuls
                  for (db, q4, half) in inj_mm.get(t, ()):
                      tiles = xkb[(db, q4)]
                      pj = psB.tile([128, 512], f32, tag="pj", bufs=2)
                      for kc in range(NDB):
                          nc.tensor.matmul(
                              pj[:, :],
                              wink[kc][:, db * 128:(db + 1) * 128],
                              tiles[kc][:, half * 512:(half + 1) * 512],
                              start=(kc == 0), stop=(kc == NDB - 1))
                      nc.vector.tensor_scalar_add(
                          kt[db][:, q4 * QC + half * 512:
                                 q4 * QC + (half + 1) * 512],
                          pj[:, :], bk_t[:, db:db + 1])
                  # scores
                  ps = psB.tile([128, QC], f32, tag="sc", bufs=2)
                  for ch in range(2):
                      nc.tensor.matmul(
                          ps[:, ch * 512:(ch + 1) * 512],
                          kt[hp][hh * 64:(hh + 1) * 64,
                                 kb * 128:(kb + 1) * 128],
                          qt[hp][hh * 64:(hh + 1) * 64,
                                 ch * 512:(ch + 1) * 512],
                          start=True, stop=True)
                  probs = bpool.tile([128, QC], bf16, tag="probs", bufs=8)
                  nc.scalar.activation(probs[:, :], ps[:, :], AF.Exp)
                  nc.vector.tensor_tensor(probs[:, :], probs[:, :],
                                          mk[kb][:, :], op=MULT)
                  norm_advance(t, norm_q)
                  # PV trailing
                  pend_pv.append((h, kb, probs))
                  if len(pend_pv) > LAG:
                      ph, pkb, pprobs = pend_pv.pop(0)
                      emit_pv(ph, pkb, pprobs)
                      if pkb == NKB - 1:
                          norm_q.append([t + P1, 1, ph, None])
                          norm_q.sort(key=lambda e: e[0])
              # drain
              t = len(steps)
              while pend_pv:
                  ph, pkb, pprobs = pend_pv.pop(0)
                  emit_pv(ph, pkb, pprobs)
                  if pkb == NKB - 1:
                      norm_q.append([t + P1, 1, ph, None])
                      norm_q.sort(key=lambda e: e[0])
              norm_advance(10 ** 9, norm_q)
              if debug_taps:
                  for hp in range(NDB):
                      tdb2 = bpool.tile([128, QC], f32, tag="dbg", bufs=2)
                      nc.vector.tensor_copy(tdb2[:, :], at4[hp][:, :])
                      nc.sync.dma_start(out=dbg_at[hp, :, :], in_=tdb2[:, :])

              # ================= PHASE C: output projection ==================
              for sb in range(8):
                  po = psB.tile([128, QC], f32, tag="sc", bufs=2,
                                name="po")[:, 0:512]
                  for hp in range(NDB):
                      nc.tensor.matmul(
                          po[:, :],
                          at4[hp][:, sb * 128:(sb + 1) * 128],
                          wo_p[hp][:, :],
                          start=(hp == 0), stop=(hp == NDB - 1))
                  osb = bpool.tile([128, D], f32, tag="osb", bufs=2)
                  nc.vector.tensor_tensor(osb[:, :], po[:, :], bob[:, :], op=ADD)
                  nc.sync.dma_start(out=outp[sb * 128:(sb + 1) * 128, :],
                                    in_=osb[:, :])

    _legalize_waits(nc)
    return nc


_program_cache = {}
_last_in_maps = None


def _get_program():
    if "nc" not in _program_cache:
        _program_cache["nc"] = _build_program()
    return _program_cache["nc"]


def kernel(query, key, value, mask, Wq, bq, Wk, bk, Wv, bv, Wo, bo, **_unused):
    query = np.asarray(query, dtype=np.float32)
    key = np.asarray(key, dtype=np.float32)
    value = np.asarray(value, dtype=np.float32)
    mask = np.asarray(mask)

    wqT = np.ascontiguousarray(np.asarray(Wq, np.float32).T).astype(np.float16)
    wkT = np.ascontiguousarray(np.asarray(Wk, np.float32).T).astype(np.float16)
    wvT = np.ascontiguousarray(np.asarray(Wv, np.float32).T).astype(ml_dtypes.bfloat16)
    woT = np.ascontiguousarray(np.asarray(Wo, np.float32).T).astype(ml_dtypes.bfloat16)
    bq_h = np.ascontiguousarray(np.asarray(bq, np.float32).reshape(NDB, 128).T)
    bk_h = np.ascontiguousarray(np.asarray(bk, np.float32).reshape(NDB, 128).T)
    # broadcast tiles: bv/bo replicated across all 128 partitions
    bvb_h = np.ascontiguousarray(
        np.broadcast_to(np.asarray(bv, np.float32).reshape(1, D), (128, D)))
    bob_h = np.ascontiguousarray(
        np.broadcast_to(np.asarray(bo, np.float32).reshape(1, D), (128, D)))

    # bf16 bits for the (0/1) mask: exact; pre-transposed per batch
    mbits = (mask != 0).astype(np.uint16) * np.uint16(0x3F80)
    mbitsT = [np.ascontiguousarray(mbits[b].T) for b in range(B)]

    xT = {}
    for b in range(B):
        xT[("q", b)] = np.ascontiguousarray(query[b].T).astype(np.float16)
        xT[("k", b)] = np.ascontiguousarray(key[b].T).astype(np.float16)
        xT[("v", b)] = np.ascontiguousarray(value[b].T).astype(ml_dtypes.bfloat16)

    in_maps = []
    for c in range(NC):
        b, qs = divmod(c, 4)
        in_maps.append({
            "xqT": np.ascontiguousarray(xT[("q", b)][:, qs * QC:(qs + 1) * QC]),
            "xkT": xT[("k", b)],
            "xvT": xT[("v", b)],
            "masktd": np.ascontiguousarray(
                mbitsT[b][:, qs * QC:(qs + 1) * QC]).view(ml_dtypes.bfloat16),
            "wqT": wqT, "wkT": wkT, "wvT": wvT, "woT": woT,
            "bq_d": bq_h, "bk_d": bk_h, "bvb_d": bvb_h, "bob_d": bob_h,
        })

    global _last_in_maps
    _last_in_maps = in_maps
    nc = _get_program()
    res = run_bass_kernel_spmd(nc, in_maps, list(range(NC)))

    out = np.empty((B, S, D), np.float32)
    for c in range(NC):
        b, qs = divmod(c, 4)
        out[b, qs * QC:(qs + 1) * QC, :] = res.results[c]["out"]
    return out
